# revision 1
# baseline (speedup 1.0000x reference)
"""BiAttn kernel for 8 TRN2 NeuronCores.

The additive score e[b,x,y] = k[b,x]@Wk + q[b,y]@Wq + b is constant along
each softmax row up to the q-term, and softmax is shift-invariant, so the
attention weights are independent of x: out[b,x,:] = sum_y p[y] v[b,y,:]
with p = softmax(q_b @ Wq). k and the bias cancel; the whole [B,X,Y]
attention collapses to one weighted average per batch, broadcast over X.

Sharding: one batch per core (pure data parallel, no collectives).
Per core: read q_b,v_b (16MB f32, SWDGE DMAs casting to bf16 inline),
write out_b (4MB bf16, host upcasts). Rel err ~3e-3 vs the 2e-2 gate.

Structure (all phases stream; DMA never idles):
- q streams first; per tile: DVE mult by Wq (stride-0 broadcast AP),
  reduction alternating ACT activation(Copy, accum_out)/DVE reduce_sum,
  then ONE ACT op applies Exp to a stride-0 broadcast view of the sq
  column and writes the [128,128] replicated stationary tile esq_b.
- PE interleaves per tile: d += esq_b@ones, c0 += esq_b@vh0 — both land
  pre-broadcast on all 128 partitions (M=128 costs the same as M=1).
- v streams in column halves; when half 0 closes, ACT scales c0 by 1/d
  and its 2MB write overlaps the half-1 read; c1 accumulates behind the
  vh1 stream, DVE scales it, leaving only the last 2MB write serial.

Measured 66-78us/NEFF fleet-noise dependent (~14us fixed NEFF overhead).
fp32 matmuls would cost two LOW_HIGH passes - everything engine-side is
bf16 except sq scalars and PSUM accumulation."""

import sys

import numpy as np

for _p in ("/opt/trn_rl_repo",):
    if _p not in sys.path:
        sys.path.insert(0, _p)

B, X, Y, H = 8, 2048, 2048, 1024
N_CORES = 8
P = 128
NT = Y // P
CHUNKS = [2, 2, 2, 2, 2, 2, 2, 1, 1]
assert sum(CHUNKS) == NT
OUT_DTYPE = "bfloat16"

_cache = {}


def _build():
    import concourse.bass as bass
    import concourse.mybir as mybir
    from concourse import bacc, tile

    f32 = mybir.dt.float32
    bf16 = mybir.dt.bfloat16
    out_dt = getattr(mybir.dt, OUT_DTYPE)

    nc = bacc.Bacc("TRN2", target_bir_lowering=False, debug=False,
                   num_devices=N_CORES, name="biattn")

    q = nc.dram_tensor("q", [Y, H], f32, kind="ExternalInput").ap()
    v = nc.dram_tensor("v", [Y, H], f32, kind="ExternalInput").ap()
    wq = nc.dram_tensor("wq", [P, H], f32, kind="ExternalInput").ap()
    out = nc.dram_tensor("out", [X, H], out_dt, kind="ExternalOutput").ap()

    q_t = q.rearrange("(n p) h -> n p h", p=P)
    v_t = v.rearrange("(n p) h -> n p h", p=P)
    out_r = out.rearrange("(t p) h -> t p h", p=P)

    with tile.TileContext(nc) as tc:
        with (
            tc.tile_pool(name="const", bufs=1) as constp,
            tc.tile_pool(name="qin", bufs=len(CHUNKS)) as qp,
            tc.tile_pool(name="vin", bufs=2 * len(CHUNKS)) as vp,
            tc.tile_pool(name="scr", bufs=3) as scr,
            tc.tile_pool(name="ebp", bufs=NT) as ebp,
            tc.tile_pool(name="small", bufs=1) as smallp,
            tc.tile_pool(name="ps_acc", bufs=1, space=bass.MemorySpace.PSUM) as psa,
        ):
            wq_b = constp.tile([P, H], bf16, tag="wq_b", name="wq_b")
            nc.gpsimd.dma_start(wq_b[:], wq)

            ones_col = constp.tile([P, 1], bf16, tag="ones_col", name="ones_col")
            nc.vector.memset(ones_col[:], 1.0)

            sq_all = smallp.tile([P, NT], f32, tag="sq_all", name="sq_all")

            ps_c0 = psa.tile([P, 512], f32, tag="ps_c0", name="ps_c0")
            ps_c1 = psa.tile([P, 512], f32, tag="ps_c1", name="ps_c1")
            ps_d = psa.tile([P, 1], f32, tag="ps_d", name="ps_d")

            starts = [sum(CHUNKS[:i]) for i in range(len(CHUNKS))]
            q_tiles = [qp.tile([P, cs * H], bf16, tag="q_sb",
                               name=f"q_sb{i}",
                               padded_shape=[P, max(CHUNKS) * H])
                       for i, cs in enumerate(CHUNKS)]
            # v half-column tiles: [P, cs*512] per (chunk, half)
            v_tiles = [[vp.tile([P, cs * 512], bf16, tag="v_bf",
                                name=f"v_bf{i}_{j}",
                                padded_shape=[P, max(CHUNKS) * 512])
                        for j in range(2)]
                       for i, cs in enumerate(CHUNKS)]

            # ---- DMA issue order: q interleaved with v-half0 (half0
            # finishes ~10us before stream end so the h0 output write has
            # a full window under the v-half1 stream), then v-half1 last
            def issue_q(i):
                s, cs = starts[i], CHUNKS[i]
                nc.gpsimd.dma_start(
                    q_tiles[i][:].rearrange("p (t h) -> p t h", t=cs),
                    q_t[s:s + cs].rearrange("n p h -> p n h"))

            def issue_v(i, j):
                s, cs = starts[i], CHUNKS[i]
                src = v_t[s:s + cs, :, j * 512:(j + 1) * 512]
                nc.gpsimd.dma_start(
                    v_tiles[i][j][:].rearrange("p (t h) -> p t h", t=cs),
                    src.rearrange("n p h -> p n h"))

            issue_q(0)
            for i in range(1, len(CHUNKS)):
                issue_q(i)
                issue_v(i - 1, 0)
            issue_v(len(CHUNKS) - 1, 0)
            for i in range(len(CHUNKS)):
                issue_v(i, 1)

            # ---- sq / esq / esq_b / d, paced with the q stream
            esq_bs = []
            yt = 0
            for ci, cs in enumerate(CHUNKS):
                q_sb = q_tiles[ci]
                sc = scr.tile([P, cs * H], bf16, tag="sc", name="sc",
                              padded_shape=[P, max(CHUNKS) * H])
                nc.vector.tensor_mul(
                    sc[:].rearrange("p (t h) -> p t h", t=cs),
                    q_sb[:].rearrange("p (t h) -> p t h", t=cs),
                    wq_b[:].unsqueeze(1).broadcast_to([P, cs, H]))
                for t in range(cs):
                    if yt % 2 == 1:
                        nc.vector.reduce_sum(
                            sq_all[:, yt:yt + 1], sc[:, t * H:(t + 1) * H],
                            axis=mybir.AxisListType.X)
                    else:
                        dump = scr.tile([P, H], bf16, tag="dump", name="dump")
                        nc.scalar.activation(
                            dump[:], sc[:, t * H:(t + 1) * H],
                            mybir.ActivationFunctionType.Copy,
                            accum_out=sq_all[:, yt:yt + 1])
                    # fused exp+broadcast: ACT reads the sq column via a
                    # stride-0 AP and writes the replicated [128,128]
                    # stationary tile directly (no DVE hop, no esq_all)
                    esq_b = ebp.tile([P, P], bf16, tag="esq_b",
                                     name=f"esq_b{yt}")
                    nc.scalar.activation(
                        esq_b[:], sq_all[:, yt:yt + 1].broadcast_to([P, P]),
                        mybir.ActivationFunctionType.Exp)
                    esq_bs.append(esq_b)
                    nc.tensor.matmul(
                        ps_d[:], esq_b[:], ones_col[:],
                        start=(yt == 0), stop=(yt == NT - 1))
                    # c0 matmul interleaved here: PE consumes the vh0
                    # stream as it arrives instead of queuing all c0 work
                    # behind the last d-matmul (program-order FIFO)
                    nc.tensor.matmul(
                        ps_c0[:], esq_b[:],
                        v_tiles[ci][0][:, t * 512:(t + 1) * 512],
                        start=(yt == 0), stop=(yt == NT - 1))
                    yt += 1

            inv_d = smallp.tile([P, 1], f32, tag="inv_d", name="inv_d")
            nc.vector.reciprocal(inv_d[:], ps_d[:])

            bc_sb = smallp.tile([P, H], out_dt, tag="bc_sb", name="bc_sb")

            # ---- half 0: c0 already accumulated in the q-phase loop;
            # scale on ACT (idle here; its sequencer is not yet issuing)
            nc.scalar.activation(
                bc_sb[:, 0:512], ps_c0[:],
                mybir.ActivationFunctionType.Copy, scale=inv_d[:])
            for t in range(NT):
                eng = nc.sync if t % 2 == 0 else nc.scalar
                eng.dma_start(out_r[t, :, 0:512], bc_sb[:, 0:512])

            # ---- half 1: accumulate as vh1 streams, scale on DVE (the
            # Scalar sequencer is busy issuing h0 output DMAs by now)
            yt = 0
            for ci, cs in enumerate(CHUNKS):
                for t in range(cs):
                    nc.tensor.matmul(
                        ps_c1[:], esq_bs[yt],
                        v_tiles[ci][1][:, t * 512:(t + 1) * 512],
                        start=(yt == 0), stop=(yt == NT - 1))
                    yt += 1
            nc.vector.tensor_scalar_mul(bc_sb[:, 512:H], ps_c1[:], inv_d[:])
            for t in range(NT):
                eng = nc.sync if t % 2 == 0 else nc.scalar
                eng.dma_start(out_r[t, :, 512:H], bc_sb[:, 512:H])
    nc.compile()
    return nc


def _get_nc():
    if "nc" not in _cache:
        _cache["nc"] = _build()
    return _cache["nc"]


def _in_maps(q, k, v, W, b):
    q = np.asarray(q, dtype=np.float32)
    v = np.asarray(v, dtype=np.float32)
    W = np.asarray(W, dtype=np.float32)
    wq = np.ascontiguousarray(np.broadcast_to(W[H:], (P, H)))
    return [
        {"q": np.ascontiguousarray(q[c]),
         "v": np.ascontiguousarray(v[c]),
         "wq": wq}
        for c in range(N_CORES)
    ]


def kernel(q, k, v, W, b):
    from concourse.bass_utils import run_bass_kernel_spmd

    nc = _get_nc()
    res = run_bass_kernel_spmd(nc, _in_maps(q, k, v, W, b),
                               core_ids=list(range(N_CORES)))
    outs = [np.asarray(res.results[c]["out"]).astype(np.float32)
            for c in range(N_CORES)]
    return np.stack(outs)



# revision 3
# speedup vs baseline: 1.5215x; 1.5215x over previous
"""BiAttn kernel for 8 TRN2 NeuronCores.

The additive score e[b,x,y] = k[b,x]@Wk + q[b,y]@Wq + b is constant along
each softmax row up to the q-term, and softmax is shift-invariant, so the
attention weights are independent of x: out[b,x,:] = sum_y p[y] v[b,y,:]
with p = softmax(q_b @ Wq). k and the bias cancel; the whole [B,X,Y]
attention collapses to one weighted average per batch, broadcast over X.

Sharding: one batch per core (pure data parallel, no collectives).
The host shards q,v per batch and rounds them to bf16 while staging (the
previous version did the same rounding inline in SWDGE cast-DMAs, paying
f32 HBM reads); the device streams 8.25MB of bf16 via the two HWDGE
rings (q on sync, v on scalar), computes p and c_b = sum_y p[y] v_b[y,:]
in f32 PSUM, and returns just the [1,H] f32 row c_b. Since out[b,x,:] is
c_b for every x, the host materializes the full [B,X,H] output by
broadcast during unshard — no 4MB/core HBM output write.

Per-tile pipeline, chasing the DMA stream: DVE tensor_tensor_reduce
fuses q*wq and the row-sum into one op (sq column), ACT applies Exp to a
stride-0 broadcast view writing the [128,128] replicated esq tile, PE
accumulates d += esq@ones and c0/c1 += esq@v-half pre-broadcast on all
128 partitions. Tail: DVE reciprocal of d overlaps the last c-matmuls,
ACT/DVE scale the two PSUM halves, one 4KB DMA out.
"""

import sys

import numpy as np

for _p in ("/opt/trn_rl_repo",):
    if _p not in sys.path:
        sys.path.insert(0, _p)

B, X, Y, H = 8, 2048, 2048, 1024
N_CORES = 8
P = 128
NT = Y // P
CHUNKS = [3, 3, 3, 3, 2, 1, 1]
assert sum(CHUNKS) == NT

_cache = {}


def _build():
    import concourse.bass as bass
    import concourse.mybir as mybir
    from concourse import bacc, tile

    f32 = mybir.dt.float32
    bf16 = mybir.dt.bfloat16

    nc = bacc.Bacc("TRN2", target_bir_lowering=False, debug=False,
                   num_devices=N_CORES, name="biattn")

    q = nc.dram_tensor("q", [Y, H], bf16, kind="ExternalInput").ap()
    v = nc.dram_tensor("v", [Y, H], bf16, kind="ExternalInput").ap()
    wq = nc.dram_tensor("wq", [P, H], bf16, kind="ExternalInput").ap()
    out = nc.dram_tensor("out", [1, H], f32, kind="ExternalOutput").ap()

    q_t = q.rearrange("(n p) h -> n p h", p=P)
    v_t = v.rearrange("(n p) h -> n p h", p=P)

    with tile.TileContext(nc) as tc:
        with (
            tc.tile_pool(name="const", bufs=1) as constp,
            tc.tile_pool(name="qin", bufs=len(CHUNKS)) as qp,
            tc.tile_pool(name="vin", bufs=len(CHUNKS)) as vp,
            tc.tile_pool(name="scr", bufs=2) as scr,
            tc.tile_pool(name="ebp", bufs=NT) as ebp,
            tc.tile_pool(name="small", bufs=1) as smallp,
            tc.tile_pool(name="ps_acc", bufs=1, space=bass.MemorySpace.PSUM) as psa,
        ):
            wq_b = constp.tile([P, H], bf16, tag="wq_b", name="wq_b")
            nc.sync.dma_start(wq_b[:], wq)

            ones_col = constp.tile([P, 1], bf16, tag="ones_col", name="ones_col")
            nc.vector.memset(ones_col[:], 1.0)

            sq_all = smallp.tile([P, NT], f32, tag="sq_all", name="sq_all")

            ps_c0 = psa.tile([P, 512], f32, tag="ps_c0", name="ps_c0")
            ps_c1 = psa.tile([P, 512], f32, tag="ps_c1", name="ps_c1")
            ps_d = psa.tile([P, 1], f32, tag="ps_d", name="ps_d")

            starts = [sum(CHUNKS[:i]) for i in range(len(CHUNKS))]
            q_tiles = [qp.tile([P, cs * H], bf16, tag="q_sb",
                               name=f"q_sb{i}",
                               padded_shape=[P, max(CHUNKS) * H])
                       for i, cs in enumerate(CHUNKS)]
            v_tiles = [vp.tile([P, cs * H], bf16, tag="v_sb",
                               name=f"v_sb{i}",
                               padded_shape=[P, max(CHUNKS) * H])
                       for i, cs in enumerate(CHUNKS)]

            # q chunks stream on the sync HWDGE ring, v chunks on the
            # scalar ring; the SDMA engines round-robin the two rings so
            # the streams advance together, with wq giving q a head start
            # so esq_t is ready when v tile t lands.
            for i, cs in enumerate(CHUNKS):
                s = starts[i]
                nc.sync.dma_start(
                    q_tiles[i][:].rearrange("p (t h) -> p t h", t=cs),
                    q_t[s:s + cs].rearrange("n p h -> p n h"))
            for i, cs in enumerate(CHUNKS):
                s = starts[i]
                nc.scalar.dma_start(
                    v_tiles[i][:].rearrange("p (t h) -> p t h", t=cs),
                    v_t[s:s + cs].rearrange("n p h -> p n h"))

            yt = 0
            for ci, cs in enumerate(CHUNKS):
                q_sb = q_tiles[ci]
                v_sb = v_tiles[ci]
                sc = scr.tile([P, cs * H], bf16, tag="sc", name="sc",
                              padded_shape=[P, max(CHUNKS) * H])
                nc.vector.tensor_mul(
                    sc[:].rearrange("p (t h) -> p t h", t=cs),
                    q_sb[:].rearrange("p (t h) -> p t h", t=cs),
                    wq_b[:].unsqueeze(1).broadcast_to([P, cs, H]))
                for t in range(cs):
                    # reduction alternates ACT copy-accum / DVE reduce_sum
                    # so neither engine falls behind the stream
                    if yt % 2 == 1:
                        nc.vector.reduce_sum(
                            sq_all[:, yt:yt + 1], sc[:, t * H:(t + 1) * H],
                            axis=mybir.AxisListType.X)
                    else:
                        dump = scr.tile([P, H], bf16, tag="dump", name="dump")
                        nc.scalar.activation(
                            dump[:], sc[:, t * H:(t + 1) * H],
                            mybir.ActivationFunctionType.Copy,
                            accum_out=sq_all[:, yt:yt + 1])
                    # fused exp+broadcast: ACT reads the sq column via a
                    # stride-0 AP and writes the replicated [128,128] tile
                    esq_b = ebp.tile([P, P], bf16, tag="esq_b",
                                     name=f"esq_b{yt}")
                    nc.scalar.activation(
                        esq_b[:], sq_all[:, yt:yt + 1].broadcast_to([P, P]),
                        mybir.ActivationFunctionType.Exp)
                    nc.tensor.matmul(
                        ps_d[:], esq_b[:], ones_col[:],
                        start=(yt == 0), stop=(yt == NT - 1))
                    nc.tensor.matmul(
                        ps_c0[:], esq_b[:], v_sb[:, t * H:t * H + 512],
                        start=(yt == 0), stop=(yt == NT - 1))
                    nc.tensor.matmul(
                        ps_c1[:], esq_b[:], v_sb[:, t * H + 512:(t + 1) * H],
                        start=(yt == 0), stop=(yt == NT - 1))
                    yt += 1

            inv_d = smallp.tile([P, 1], f32, tag="inv_d", name="inv_d")
            nc.vector.reciprocal(inv_d[:], ps_d[:])

            # out rows are identical across partitions; scale partition 0
            # of each PSUM half (ACT and DVE in parallel) and ship 4KB.
            bc_sb = smallp.tile([P, H], f32, tag="bc_sb", name="bc_sb")
            nc.scalar.activation(
                bc_sb[0:1, 0:512], ps_c0[0:1, :],
                mybir.ActivationFunctionType.Copy, scale=inv_d[0:1])
            nc.vector.tensor_scalar_mul(
                bc_sb[0:1, 512:H], ps_c1[0:1, :], inv_d[0:1])
            nc.sync.dma_start(out, bc_sb[0:1, :])
    nc.compile()
    return nc


def _get_nc():
    if "nc" not in _cache:
        _cache["nc"] = _build()
    return _cache["nc"]


def _in_maps(q, k, v, W, b):
    import ml_dtypes

    bf = ml_dtypes.bfloat16
    q = np.asarray(q)
    v = np.asarray(v)
    W = np.asarray(W, dtype=np.float32)
    wq = np.ascontiguousarray(np.broadcast_to(W[H:].astype(bf), (P, H)))
    return [
        {"q": np.ascontiguousarray(q[c]).astype(bf),
         "v": np.ascontiguousarray(v[c]).astype(bf),
         "wq": wq}
        for c in range(N_CORES)
    ]


def kernel(q, k, v, W, b):
    from concourse.bass_utils import run_bass_kernel_spmd

    nc = _get_nc()
    res = run_bass_kernel_spmd(nc, _in_maps(q, k, v, W, b),
                               core_ids=list(range(N_CORES)))
    c_rows = np.stack([
        np.asarray(res.results[c]["out"], dtype=np.float32).reshape(H)
        for c in range(N_CORES)
    ])
    return np.ascontiguousarray(
        np.broadcast_to(c_rows[:, None, :], (B, X, H)))
